# revision 23
# baseline (speedup 1.0000x reference)
"""Trainium2 Bass kernel for nn_AgentModel_30717606101642.

Computes, for x (B,256) through two small MLPs:
  pdf (B,101) — mixture of [pi0, beta-bin-masses*pi2, pi1], normalized
  v   (B,)    — sigmoid head

Key algorithmic idea: the reference's 41-step Lentz continued fraction for
the beta CDF is only used through *differences* cdf[j+1]-cdf[j].  Those are
bin integrals of the beta density, which we evaluate directly with per-bin
Gauss-Legendre quadrature:
    bin_j = sum_q exp((a-1)*log t_jq + (b-1)*log(1-t_jq) + log w_jq - logB)
The exponent is a K=3 matmul (PE), the exp is one ACT op per row-tile, and
the q-sum is a DVE reduce.  The first/last bins (integrable singularities)
use a 6-term analytic series.  Validated to ~1e-6 absmax against the
reference on the full batch.

Data parallel across 8 cores: batch sharded, weights replicated.
"""
import sys
import os

for _p in ("/opt/trn_rl_repo", "/root/.axon_site/_ro/trn_rl_repo"):
    if os.path.isdir(_p) and _p not in sys.path:
        sys.path.append(_p)

import numpy as np
import concourse.bass as bass
import concourse.bacc as bacc
import concourse.mybir as mybir
import concourse.tile as tile
from concourse.bass_utils import run_bass_kernel_spmd

f32 = mybir.dt.float32
f32r = mybir.dt.float32r
AF = mybir.ActivationFunctionType
ALU = mybir.AluOpType
AX = mybir.AxisListType

N_CORES = 8
B_FULL = 65536
F = 256
NGRID = 100
NQ = 2          # Gauss-Legendre points per interior bin
NB = 97         # interior bins (pdf cols 2..98); bins 0 and 98 via series
EW = NB * NQ    # exponent width per row-tile
NSER = 5        # edge-series terms
USE_F32R = True  # matmul dtype knob

HALF_LN2PI = 0.9189385332046727
STIR1, STIR2, STIR3 = 1.0 / 12.0, -1.0 / 360.0, 1.0 / 1260.0


def host_constants():
    grid = np.linspace(0.0, 1.0, NGRID, dtype=np.float32).astype(np.float64)
    xg, wg = np.polynomial.legendre.leggauss(NQ)
    L = np.zeros((3, NB, NQ))
    for j in range(NB):
        lo, hi = grid[j + 1], grid[j + 2]
        mid, half = 0.5 * (lo + hi), 0.5 * (hi - lo)
        t = mid + half * xg
        L[0, j] = np.log(t)
        L[1, j] = np.log1p(-t)
        L[2, j] = np.log(half * wg)
    # recenter rows to shrink f32r rounding error; compensated in the bias
    shifts = L.reshape(3, -1).mean(-1)
    Lc = np.zeros((8, EW), np.float32)
    Lc[0:3] = (L - shifts[:, None, None]).reshape(3, EW).astype(np.float32)
    x1 = float(grid[1])
    x1b = float(1.0 - np.float32(grid[98]))
    return Lc, shifts, x1, x1b


_L_CONST, _L_SHIFTS, _X1, _X1B = host_constants()


def build(B_core, CH=1024):
    """Emit the per-core program. B_core rows; CH-row chunks for the MLP."""
    assert B_core % CH == 0 and CH % 512 == 0
    NCH = B_core // CH
    NT = B_core // 128          # row-tiles (also head free-dim width)
    TPC = CH // 128             # row-tiles per chunk
    FPC = CH // 128             # head cols per chunk (row = 128*f + p)
    mdt = f32r if USE_F32R else f32

    nc = bacc.Bacc("TRN2", target_bir_lowering=False, debug=False)

    x_d = nc.dram_tensor("x", [B_core, F], f32, kind="ExternalInput")
    wds = {}
    for nm, shp in [("pW1", [F, 256]), ("pW2", [256, 256]), ("pW3", [256, 5]),
                    ("vW1", [F, 256]), ("vW2", [256, 256]), ("vW3", [256, 1])]:
        wds[nm] = nc.dram_tensor(nm, shp, mdt, kind="ExternalInput")
    bds = {}
    for nm, shp in [("pb1", [256]), ("pb2", [256]), ("pb3", [5]),
                    ("vb1", [256]), ("vb2", [256]), ("vb3", [1])]:
        bds[nm] = nc.dram_tensor(nm, shp, f32, kind="ExternalInput")
    L_d = nc.dram_tensor("Lmat", [8, EW], mdt, kind="ExternalInput")
    ones_d = nc.dram_tensor("cfpad", [6, B_core], mdt, kind="ExternalInput")
    id_d = nc.dram_tensor("ident", [128, 128], f32, kind="ExternalInput")
    th_scr = nc.dram_tensor("th_scr", [6, B_core], f32)
    cf_scr = nc.dram_tensor("cf_scr", [2, B_core], f32)
    pdf_d = nc.dram_tensor("pdf", [B_core, 101], f32, kind="ExternalOutput")
    v_d = nc.dram_tensor("v", [B_core], f32, kind="ExternalOutput")

    with tile.TileContext(nc) as tc:
      with tc.tile_pool(name="const", bufs=1) as cp:
        # ---- persistent SBUF ----
        w1 = cp.tile([128, 2, 2, 128], mdt)   # [k, m] tiles of pW1
        w2 = cp.tile([128, 2, 2, 128], mdt)
        vw1 = cp.tile([128, 2, 2, 128], mdt)
        vw2 = cp.tile([128, 2, 2, 128], mdt)
        w3 = cp.tile([128, 2, 8], mdt)        # [k, {theta0..4, v, pad}]
        b1 = cp.tile([128, 2], f32)
        b2 = cp.tile([128, 2], f32)
        vb1 = cp.tile([128, 2], f32)
        vb2 = cp.tile([128, 2], f32)
        b3 = cp.tile([5, 1], f32)
        b3v = cp.tile([1, 1], f32)
        Ls = cp.tile([8, EW], mdt)
        idn = cp.tile([128, 128], f32)
        coefT = cp.tile([8, B_core], mdt)
        bins = cp.tile([128, NT * NB], f32)
        th = [cp.tile([128, NT], f32, name=f"th{j}") for j in range(6)]   # theta comps, head layout
        # head-phase values
        ha = cp.tile([128, NT], f32)
        hb = cp.tile([128, NT], f32)
        hnlB = cp.tile([128, NT], f32)     # -logB - shift-compensation
        he = [cp.tile([128, NT], f32, name=f"he{j}") for j in range(3)]
        hE1 = cp.tile([128, NT], f32)
        hE99 = cp.tile([128, NT], f32)
        hsum = cp.tile([128, NT], f32)
        hw2 = cp.tile([128, NT], f32)
        hp0 = cp.tile([128, NT], f32)
        hp1 = cp.tile([128, NT], f32)
        s = [cp.tile([128, NT], f32, name=f"hs{j}") for j in range(8)]    # scratch
        s2 = [cp.tile([128, NT], f32, name=f"hs2_{j}") for j in range(6)]
        s3 = [cp.tile([128, NT], f32, name=f"hs3_{j}") for j in range(6)]
        s4 = [cp.tile([128, NT], f32, name=f"hs4_{j}") for j in range(6)]
        s5 = [cp.tile([128, NT], f32, name=f"hs5_{j}") for j in range(6)]
        c_one = cp.tile([128, 1], f32)
        c_eight = cp.tile([128, 1], f32)
        nc.vector.memset(c_one[:], 1.0)
        nc.vector.memset(c_eight[:], 8.0)

        # ---- weight/const loads ----
        for wt, dname in [(w1, "pW1"), (w2, "pW2"), (vw1, "vW1"), (vw2, "vW2")]:
            eng = nc.sync if wt is w1 else nc.gpsimd
            for k in range(2):
                for m in range(2):
                    eng.dma_start(wt[:, k, m, :],
                                  wds[dname][128 * k:128 * (k + 1),
                                             128 * m:128 * (m + 1)])
        for k in range(2):
            nc.gpsimd.dma_start(w3[:, k, 0:5], wds["pW3"][128 * k:128 * (k + 1), :])
            nc.gpsimd.dma_start(w3[:, k, 5:6], wds["vW3"][128 * k:128 * (k + 1), :])
        for bt, dname in [(b1, "pb1"), (b2, "pb2"), (vb1, "vb1"), (vb2, "vb2")]:
            for m in range(2):
                nc.gpsimd.dma_start(
                    bt[:, m:m + 1],
                    bds[dname][128 * m:128 * (m + 1)].rearrange("(p o) -> p o", o=1))
        nc.gpsimd.dma_start(b3[:], bds["pb3"][:].rearrange("(p o) -> p o", o=1))
        nc.gpsimd.dma_start(b3v[:], bds["vb3"][:].rearrange("(p o) -> p o", o=1))
        nc.sync.dma_start(Ls[:], L_d[:])
        nc.gpsimd.dma_start(idn[:], id_d[:])
        nc.sync.dma_start(coefT[2:8, :], ones_d[:])

        # ---- phase A: transpose x + MLPs, chunked over rows ----
        with (
            tc.tile_pool(name="xl", bufs=3) as xlp,
            tc.tile_pool(name="hc", bufs=2) as hp,
            tc.tile_pool(name="tc1", bufs=2) as tcp,
            tc.tile_pool(name="tp", bufs=1, space=bass.MemorySpace.PSUM) as tpp,
            tc.tile_pool(name="mm", bufs=3, space=bass.MemorySpace.PSUM) as mmp,
        ):
            for c in range(NCH):
                xT = hp.tile([128, 2, CH], mdt, tag="xT")
                for g in range(TPC // 4):
                    t0 = TPC * c + 4 * g
                    xg = xlp.tile([128, 4, F], f32, tag="xl")
                    nc.sync.dma_start(
                        xg[:],
                        x_d[128 * t0:128 * t0 + 512, :].rearrange(
                            "(u p) f -> p u f", p=128))
                    for u in range(4):
                        i = 4 * g + u
                        tp = tpp.tile([128, 2, 128], f32, tag="tp")
                        for kb in range(2):
                            nc.tensor.transpose(
                                tp[:, kb, :],
                                xg[:, u, 128 * kb:128 * (kb + 1)], idn[:])
                        nc.vector.tensor_copy(xT[:, :, 128 * i:128 * (i + 1)],
                                              tp[:])

                def layer(rhs, wt, bt, tag, act=True):
                    out = hp.tile([128, 2, CH], mdt, tag=tag)
                    for m in range(2):
                        ps = mmp.tile([128, CH], f32, tag="mm")
                        for k in range(2):
                            for n in range(CH // 512):
                                nc.tensor.matmul(
                                    ps[:, 512 * n:512 * (n + 1)],
                                    wt[:, k, m, :],
                                    rhs[:, k, 512 * n:512 * (n + 1)],
                                    start=(k == 0), stop=(k == 1))
                        if m == 0:
                            nc.scalar.activation(out[:, m, :], ps[:], AF.Relu,
                                                 bias=bt[:, m:m + 1])
                        else:
                            nc.vector.tensor_scalar(out[:, m, :], ps[:],
                                                    bt[:, m:m + 1], 0.0,
                                                    ALU.add, ALU.max)
                    return out

                h1 = layer(xT, w1, b1, "h1")
                h2 = layer(h1, w2, b2, "h2")
                g1 = layer(xT, vw1, vb1, "g1")
                g2 = layer(g1, vw2, vb2, "g2")

                thc = tcp.tile([33, CH], f32, tag="thc")
                ps3 = mmp.tile([128, CH], f32, tag="mm")
                vp3 = mmp.tile([128, CH], f32, tag="mm")
                for n in range(CH // 512):
                    for k in range(2):
                        nc.tensor.matmul(ps3[0:5, 512 * n:512 * (n + 1)],
                                         w3[:, k, 0:5],
                                         h2[:, k, 512 * n:512 * (n + 1)],
                                         start=(k == 0), stop=(k == 1))
                    for k in range(2):
                        nc.tensor.matmul(vp3[0:2, 512 * n:512 * (n + 1)],
                                         w3[:, k, 5:7],
                                         g2[:, k, 512 * n:512 * (n + 1)],
                                         start=(k == 0), stop=(k == 1))
                nc.scalar.activation(thc[0:5, :], ps3[0:5, :],
                                     AF.Identity, bias=b3[:])
                nc.scalar.activation(thc[32:33, :], vp3[0:1, :],
                                     AF.Identity, bias=b3v[:])
                # theta chunk -> DRAM scratch (contiguous)
                nc.sync.dma_start(th_scr[0:5, CH * c:CH * (c + 1)], thc[0:5, :])
                nc.sync.dma_start(th_scr[5:6, CH * c:CH * (c + 1)],
                                  thc[32:33, :])
                # reshape into head layout (row = 128*f + p) via PE transpose
                for j in range(6):
                    tsc = xlp.tile([TPC, 128], f32, tag="tsc")
                    nc.sync.dma_start(
                        tsc[:],
                        th_scr[j, CH * c:CH * (c + 1)].rearrange(
                            "(f p) -> f p", p=128))
                    pst = tpp.tile([128, TPC], f32, tag="pst")
                    nc.tensor.transpose(pst[:], tsc[:], idn[0:TPC, 0:TPC])
                    nc.vector.tensor_copy(th[j][:, FPC * c:FPC * (c + 1)],
                                          pst[:])

        # ---- phase B1: heads (all tensors (128, NT) fp32) ----
        V = nc.vector
        S = nc.scalar

        def softplus(dst, src, w):
            V.tensor_scalar(w[0][:], src[:], -5.0, None, ALU.max)      # c
            S.activation(w[1][:], w[0][:], AF.Abs)
            S.activation(w[1][:], w[1][:], AF.Exp, scale=-1.0)         # exp(-|c|)
            S.activation(w[1][:], w[1][:], AF.Ln, bias=c_one[:])       # log1p
            V.tensor_scalar(w[2][:], w[0][:], 0.0, None, ALU.max)      # relu(c)
            V.tensor_add(dst[:], w[1][:], w[2][:])

        softplus(ha, th[3], s2)
        softplus(hb, th[4], s3)

        # v head: sigmoid = 1/(1+exp(-x)); stored, written out in phase C1
        hvs = cp.tile([128, NT], f32)
        S.activation(s[0][:], th[5][:], AF.Exp, scale=-1.0)
        V.tensor_scalar(s[0][:], s[0][:], 1.0, None, ALU.add)
        V.reciprocal(hvs[:], s[0][:])

        # softmax numerators
        V.tensor_max(s[0][:], th[0][:], th[1][:])
        V.tensor_max(s[0][:], s[0][:], th[2][:])
        for j in range(3):
            V.tensor_sub(s[1][:], th[j][:], s[0][:])
            S.activation(he[j][:], s[1][:], AF.Exp)

        def gammaln(dst, z, w):
            # z>0; gammaln via 8-shift + Stirling.  dst may not alias z.
            V.tensor_scalar(w[0][:], z[:], 7.0, None, ALU.add)
            V.tensor_mul(w[0][:], w[0][:], z[:])                       # w=z^2+7z
            V.tensor_scalar(w[1][:], w[0][:], 6.0, None, ALU.add)
            V.tensor_mul(w[1][:], w[1][:], w[0][:])                    # w(w+6)
            V.tensor_scalar(w[2][:], w[0][:], 10.0, None, ALU.add)
            V.tensor_mul(w[1][:], w[1][:], w[2][:])
            V.tensor_scalar(w[2][:], w[0][:], 12.0, None, ALU.add)
            V.tensor_mul(w[1][:], w[1][:], w[2][:])                    # prod
            S.activation(w[1][:], w[1][:], AF.Ln)                      # logprod
            S.activation(w[2][:], z[:], AF.Ln, bias=c_eight[:])        # log(z+8)
            V.tensor_scalar(w[3][:], z[:], 8.0, None, ALU.add)
            V.reciprocal_approx_fast(w[4][:], w[3][:])                 # r=1/(z+8)
            V.tensor_mul(w[3][:], w[4][:], w[4][:])                    # r2
            V.tensor_scalar(w[5][:], w[3][:], STIR3, STIR2, ALU.mult, ALU.add)
            V.tensor_mul(w[5][:], w[5][:], w[3][:])
            V.tensor_scalar(w[5][:], w[5][:], STIR1, None, ALU.add)
            V.tensor_mul(w[5][:], w[5][:], w[4][:])                    # ser
            V.tensor_scalar(w[3][:], z[:], 7.5, None, ALU.add)
            V.tensor_mul(w[3][:], w[3][:], w[2][:])                    # (u-.5)logu
            V.tensor_sub(w[3][:], w[3][:], z[:])
            V.tensor_add(w[3][:], w[3][:], w[5][:])
            V.tensor_scalar(w[3][:], w[3][:], HALF_LN2PI - 8.0, None, ALU.add)
            V.tensor_sub(dst[:], w[3][:], w[1][:])

        ga, gb, gab = s[6], s[7], hnlB
        gammaln(ga, ha, s2)
        gammaln(gb, hb, s3)
        V.tensor_add(hw2[:], ha[:], hb[:])   # hw2 free until phase B2
        gammaln(gab, hw2, s4)
        # hnlB (=gab tile) := gab - ga - gb  == -logB
        V.tensor_sub(hnlB[:], hnlB[:], ga[:])
        V.tensor_sub(hnlB[:], hnlB[:], gb[:])

        def edge(dst, z, zq, x1, w):
            # dst = x1^z / B * sum_{n<NSER} c_n(zq) x1^n / (z+n)
            lnx1 = float(np.log(x1))
            V.reciprocal_approx_fast(w[0][:], z[:])
            V.tensor_copy(w[1][:], w[0][:])                            # S
            first = True
            for n in range(1, NSER):
                V.tensor_scalar(w[2][:], zq[:], -x1 / n, x1, ALU.mult, ALU.add)
                if first:
                    V.tensor_copy(w[3][:], w[2][:])
                    first = False
                else:
                    V.tensor_mul(w[3][:], w[3][:], w[2][:])            # c_n
                V.tensor_scalar(w[4][:], z[:], float(n), None, ALU.add)
                V.reciprocal_approx_fast(w[0][:], w[4][:])
                V.tensor_mul(w[4][:], w[3][:], w[0][:])
                V.tensor_add(w[1][:], w[1][:], w[4][:])
            V.tensor_scalar(w[2][:], z[:], lnx1, None, ALU.mult)
            V.tensor_add(w[2][:], w[2][:], hnlB[:])
            S.activation(w[2][:], w[2][:], AF.Exp)
            V.tensor_mul(dst[:], w[2][:], w[1][:])

        edge(hE1, ha, hb, _X1, s2)
        edge(hE99, hb, ha, _X1B, s3)

        # coefT rows 0,1 = a-1, b-1 (head -> tile-major via cast DMA);
        # bias compensation for the recentered L rows:
        #   hnlB += sh0*(a-1) + sh1*(b-1) + sh2
        sh0, sh1, sh2 = (float(v) for v in _L_SHIFTS)
        V.tensor_scalar(s[0][:], ha[:], 1.0, None, ALU.subtract)
        V.tensor_scalar(s[1][:], hb[:], 1.0, None, ALU.subtract)
        V.tensor_scalar(s[2][:], s[0][:], sh0, sh2, ALU.mult, ALU.add)
        V.tensor_add(hnlB[:], hnlB[:], s[2][:])
        V.tensor_scalar(s[2][:], s[1][:], sh1, None, ALU.mult)
        V.tensor_add(hnlB[:], hnlB[:], s[2][:])

        # ---- phase C1: quadrature bins per row-tile ----
        with (
            tc.tile_pool(name="ep", bufs=4, space=bass.MemorySpace.PSUM) as epp,
            tc.tile_pool(name="gp", bufs=3) as gpp,
        ):
            # head->tile-major transposes: coefT rows (a-1, b-1) and v output
            for r, src in ((0, s[0]), (1, s[1])):
                pc = epp.tile([NT, 128], f32, tag="pc")
                nc.tensor.transpose(pc[:], src[:], idn[:])
                sc = gpp.tile([NT, 128], f32, tag="sc")
                nc.vector.tensor_copy(sc[:], pc[:])
                nc.sync.dma_start(
                    cf_scr[r, :].rearrange("(f p) -> f p", p=128), sc[:])
            nc.gpsimd.dma_start(coefT[0:2, :], cf_scr[:])
            pv = epp.tile([NT, 128], f32, tag="pc")
            nc.tensor.transpose(pv[:], hvs[:], idn[:])
            sv = gpp.tile([NT, 128], f32, tag="sc")
            nc.vector.tensor_copy(sv[:], pv[:])
            nc.sync.dma_start(v_d[:].rearrange("(f p) -> f p", p=128), sv[:])
            for t in range(NT):
                ps = epp.tile([128, EW], f32, tag="E")
                nc.tensor.matmul(ps[:], coefT[:, 128 * t:128 * (t + 1)], Ls[:],
                                 start=True, stop=True)
                G = gpp.tile([128, EW], f32, tag="G")
                nc.scalar.activation(G[:], ps[:], AF.Exp,
                                     bias=hnlB[:, t:t + 1],
                                     accum_out=hsum[:, t:t + 1])
                nc.vector.tensor_reduce(
                    bins[:, NB * t:NB * (t + 1)],
                    G[:].rearrange("p (j q) -> p j q", q=NQ),
                    axis=AX.X, op=ALU.add)

        # ---- phase B2: normalization factors ----
        V.tensor_add(s[0][:], hsum[:], hE1[:])
        V.tensor_add(s[0][:], s[0][:], hE99[:])           # T
        V.tensor_mul(s[1][:], he[2][:], s[0][:])
        V.tensor_add(s[1][:], s[1][:], he[0][:])
        V.tensor_add(s[1][:], s[1][:], he[1][:])          # denom
        V.reciprocal(s[2][:], s[1][:])                    # dr
        V.tensor_mul(hw2[:], he[2][:], s[2][:])
        V.tensor_mul(hp0[:], he[0][:], s[2][:])
        V.tensor_mul(hp1[:], he[1][:], s[2][:])
        V.tensor_mul(hE1[:], hE1[:], hw2[:])
        V.tensor_mul(hE99[:], hE99[:], hw2[:])
        # interleave [p0, E1n, E99n, p1] per tile for cheap C3 copies
        edg = cp.tile([128, NT, 4], f32)
        V.tensor_copy(edg[:, :, 0], hp0[:])
        V.tensor_copy(edg[:, :, 1], hE1[:])
        V.tensor_copy(edg[:, :, 2], hE99[:])
        V.tensor_copy(edg[:, :, 3], hp1[:])

        # ---- phase C3: assemble + store pdf ----
        with tc.tile_pool(name="op", bufs=6) as opp:
            for t in range(NT):
                O = opp.tile([128, 101], f32, tag="O")
                nc.gpsimd.tensor_scalar(O[:, 2:99], bins[:, NB * t:NB * (t + 1)],
                                        hw2[:, t:t + 1], None, ALU.mult)
                nc.vector.tensor_copy(O[:, 0:2], edg[:, t, 0:2])
                nc.vector.tensor_copy(O[:, 99:101], edg[:, t, 2:4])
                nc.sync.dma_start(pdf_d[128 * t:128 * (t + 1), :], O[:])

    nc.compile()
    return nc


_NC_CACHE = {}


def _get_nc(B_core):
    if B_core not in _NC_CACHE:
        _NC_CACHE[B_core] = build(B_core)
    return _NC_CACHE[B_core]


def make_in_maps(inputs, B_core, n_cores):
    cfpad = np.zeros((6, B_core), np.float32)
    cfpad[0] = 1.0
    ident = np.eye(128, dtype=np.float32)
    maps = []
    for c in range(n_cores):
        m = {"x": np.ascontiguousarray(inputs["x"][B_core * c:B_core * (c + 1)]),
             "Lmat": _L_CONST, "cfpad": cfpad, "ident": ident}
        for nm in ("pW1", "pb1", "pW2", "pb2", "pW3", "pb3",
                   "vW1", "vb1", "vW2", "vb2", "vW3", "vb3"):
            m[nm] = np.asarray(inputs[nm], np.float32)
        maps.append(m)
    return maps


def kernel(**inputs):
    B = inputs["x"].shape[0]
    B_core = B // N_CORES
    nc = _get_nc(B_core)
    maps = make_in_maps(inputs, B_core, N_CORES)
    res = run_bass_kernel_spmd(nc, maps, list(range(N_CORES))).results
    pdf = np.concatenate([r["pdf"] for r in res], axis=0)
    v = np.concatenate([r["v"] for r in res], axis=0)
    return pdf, v


# revision 24
# speedup vs baseline: 1.4137x; 1.4137x over previous
"""Trainium2 Bass kernel for nn_AgentModel_30717606101642.

Computes, for x (B,256) through two small MLPs:
  pdf (B,101) — mixture of [pi0, beta-bin-masses*pi2, pi1], normalized
  v   (B,)    — sigmoid head

Key algorithmic idea: the reference's 41-step Lentz continued fraction for
the beta CDF is only used through *differences* cdf[j+1]-cdf[j].  Those are
bin integrals of the beta density, which we evaluate directly with per-bin
Gauss-Legendre quadrature:
    bin_j = sum_q exp((a-1)*log t_jq + (b-1)*log(1-t_jq) + log w_jq - logB)
The exponent is a K=3 matmul (PE), the exp is one ACT op per row-tile, and
the q-sum is a DVE reduce.  The first/last bins (integrable singularities)
use a 6-term analytic series.  Validated to ~1e-6 absmax against the
reference on the full batch.

Data parallel across 8 cores: batch sharded, weights replicated.
"""
import sys
import os

for _p in ("/opt/trn_rl_repo", "/root/.axon_site/_ro/trn_rl_repo"):
    if os.path.isdir(_p) and _p not in sys.path:
        sys.path.append(_p)

import numpy as np
import concourse.bass as bass
import concourse.bacc as bacc
import concourse.mybir as mybir
import concourse.tile as tile
from concourse.bass_utils import run_bass_kernel_spmd

f32 = mybir.dt.float32
f32r = mybir.dt.float32r
AF = mybir.ActivationFunctionType
ALU = mybir.AluOpType
AX = mybir.AxisListType

N_CORES = 8
B_FULL = 65536
F = 256
NGRID = 100
NQ = 2          # Gauss-Legendre points per interior bin
NB = 97         # interior bins (pdf cols 2..98); bins 0 and 98 via series
EW = NB * NQ    # exponent width per row-tile
NSER = 5        # edge-series terms
USE_F32R = True  # matmul dtype knob

HALF_LN2PI = 0.9189385332046727
STIR1, STIR2, STIR3 = 1.0 / 12.0, -1.0 / 360.0, 1.0 / 1260.0


def host_constants():
    grid = np.linspace(0.0, 1.0, NGRID, dtype=np.float32).astype(np.float64)
    xg, wg = np.polynomial.legendre.leggauss(NQ)
    L = np.zeros((3, NB, NQ))
    for j in range(NB):
        lo, hi = grid[j + 1], grid[j + 2]
        mid, half = 0.5 * (lo + hi), 0.5 * (hi - lo)
        t = mid + half * xg
        L[0, j] = np.log(t)
        L[1, j] = np.log1p(-t)
        L[2, j] = np.log(half * wg)
    # recenter rows to shrink f32r rounding error; compensated in the bias
    shifts = L.reshape(3, -1).mean(-1)
    Lc = np.zeros((8, EW), np.float32)
    Lc[0:3] = (L - shifts[:, None, None]).reshape(3, EW).astype(np.float32)
    x1 = float(grid[1])
    x1b = float(1.0 - np.float32(grid[98]))
    return Lc, shifts, x1, x1b


_L_CONST, _L_SHIFTS, _X1, _X1B = host_constants()


def build(B_core, CH=1024):
    """Emit the per-core program. B_core rows; CH-row chunks for the MLP."""
    assert B_core % CH == 0 and CH % 512 == 0
    NCH = B_core // CH
    NT = B_core // 128          # row-tiles (also head free-dim width)
    TPC = CH // 128             # row-tiles per chunk
    FPC = CH // 128             # head cols per chunk (row = 128*f + p)
    mdt = f32r if USE_F32R else f32

    nc = bacc.Bacc("TRN2", target_bir_lowering=False, debug=False)

    x_d = nc.dram_tensor("x", [B_core, F], f32, kind="ExternalInput")
    wds = {}
    for nm, shp in [("pW1", [F, 256]), ("pW2", [256, 256]), ("pW3", [256, 5]),
                    ("vW1", [F, 256]), ("vW2", [256, 256]), ("vW3", [256, 1])]:
        wds[nm] = nc.dram_tensor(nm, shp, mdt, kind="ExternalInput")
    bds = {}
    for nm, shp in [("pb1", [256]), ("pb2", [256]), ("pb3", [5]),
                    ("vb1", [256]), ("vb2", [256]), ("vb3", [1])]:
        bds[nm] = nc.dram_tensor(nm, shp, f32, kind="ExternalInput")
    L_d = nc.dram_tensor("Lmat", [8, EW], mdt, kind="ExternalInput")
    ones_d = nc.dram_tensor("cfpad", [6, B_core], mdt, kind="ExternalInput")
    id_d = nc.dram_tensor("ident", [128, 128], f32, kind="ExternalInput")
    th_scr = nc.dram_tensor("th_scr", [6, B_core], f32)
    cf_scr = nc.dram_tensor("cf_scr", [2, B_core], f32)
    pdf_d = nc.dram_tensor("pdf", [B_core, 101], f32, kind="ExternalOutput")
    v_d = nc.dram_tensor("v", [B_core], f32, kind="ExternalOutput")

    with tile.TileContext(nc) as tc:
      with tc.tile_pool(name="const", bufs=1) as cp:
        # ---- persistent SBUF ----
        w1 = cp.tile([128, 2, 2, 128], mdt)   # [k, m] tiles of pW1
        w2 = cp.tile([128, 2, 2, 128], mdt)
        vw1 = cp.tile([128, 2, 2, 128], mdt)
        vw2 = cp.tile([128, 2, 2, 128], mdt)
        w3 = cp.tile([128, 2, 8], mdt)        # [k, {theta0..4, v, pad}]
        b1 = cp.tile([128, 2], f32)
        b2 = cp.tile([128, 2], f32)
        vb1 = cp.tile([128, 2], f32)
        vb2 = cp.tile([128, 2], f32)
        b3 = cp.tile([5, 1], f32)
        b3v = cp.tile([1, 1], f32)
        Ls = cp.tile([8, EW], mdt)
        idn = cp.tile([128, 128], f32)
        coefT = cp.tile([8, B_core], mdt)
        bins = cp.tile([128, NT * NB], f32)
        th = [cp.tile([128, NT], f32, name=f"th{j}") for j in range(6)]   # theta comps, head layout
        # head-phase values
        ha = cp.tile([128, NT], f32)
        hb = cp.tile([128, NT], f32)
        hnlB = cp.tile([128, NT], f32)     # -logB - shift-compensation
        he = [cp.tile([128, NT], f32, name=f"he{j}") for j in range(3)]
        hE1 = cp.tile([128, NT], f32)
        hE99 = cp.tile([128, NT], f32)
        hsum = cp.tile([128, NT], f32)
        hw2 = cp.tile([128, NT], f32)
        hp0 = cp.tile([128, NT], f32)
        hp1 = cp.tile([128, NT], f32)
        s = [cp.tile([128, NT], f32, name=f"hs{j}") for j in range(8)]    # scratch
        s2 = [cp.tile([128, NT], f32, name=f"hs2_{j}") for j in range(6)]
        s3 = [cp.tile([128, NT], f32, name=f"hs3_{j}") for j in range(6)]
        s4 = [cp.tile([128, NT], f32, name=f"hs4_{j}") for j in range(6)]
        s5 = [cp.tile([128, NT], f32, name=f"hs5_{j}") for j in range(6)]
        c_one = cp.tile([128, 1], f32)
        c_eight = cp.tile([128, 1], f32)
        nc.vector.memset(c_one[:], 1.0)
        nc.vector.memset(c_eight[:], 8.0)

        # ---- weight/const loads ----
        for wt, dname in [(w1, "pW1"), (w2, "pW2"), (vw1, "vW1"), (vw2, "vW2")]:
            eng = nc.sync if wt is w1 else nc.gpsimd
            for k in range(2):
                for m in range(2):
                    eng.dma_start(wt[:, k, m, :],
                                  wds[dname][128 * k:128 * (k + 1),
                                             128 * m:128 * (m + 1)])
        for k in range(2):
            nc.gpsimd.dma_start(w3[:, k, 0:5], wds["pW3"][128 * k:128 * (k + 1), :])
            nc.gpsimd.dma_start(w3[:, k, 5:6], wds["vW3"][128 * k:128 * (k + 1), :])
        for bt, dname in [(b1, "pb1"), (b2, "pb2"), (vb1, "vb1"), (vb2, "vb2")]:
            for m in range(2):
                nc.gpsimd.dma_start(
                    bt[:, m:m + 1],
                    bds[dname][128 * m:128 * (m + 1)].rearrange("(p o) -> p o", o=1))
        nc.gpsimd.dma_start(b3[:], bds["pb3"][:].rearrange("(p o) -> p o", o=1))
        nc.gpsimd.dma_start(b3v[:], bds["vb3"][:].rearrange("(p o) -> p o", o=1))
        nc.sync.dma_start(Ls[:], L_d[:])
        nc.gpsimd.dma_start(idn[:], id_d[:])
        nc.sync.dma_start(coefT[2:8, :], ones_d[:])

        # ---- phase A: transpose x + MLPs, chunked over rows ----
        with (
            tc.tile_pool(name="xl", bufs=3) as xlp,
            tc.tile_pool(name="hc", bufs=2) as hp,
            tc.tile_pool(name="tc1", bufs=2) as tcp,
            tc.tile_pool(name="tp", bufs=2, space=bass.MemorySpace.PSUM) as tpp,
            tc.tile_pool(name="mm", bufs=2, space=bass.MemorySpace.PSUM) as mmp,
            tc.tile_pool(name="thp", bufs=1, space=bass.MemorySpace.PSUM) as thp,
        ):
            for c in range(NCH):
                xT = hp.tile([128, 2, CH], mdt, tag="xT")
                for g in range(TPC // 4):
                    t0 = TPC * c + 4 * g
                    xg = xlp.tile([128, 4, F], f32, tag="xl")
                    nc.sync.dma_start(
                        xg[:],
                        x_d[128 * t0:128 * t0 + 512, :].rearrange(
                            "(u p) f -> p u f", p=128))
                    for u in range(4):
                        i = 4 * g + u
                        tp = tpp.tile([128, 2, 128], f32, tag="tp")
                        for kb in range(2):
                            nc.tensor.transpose(
                                tp[:, kb, :],
                                xg[:, u, 128 * kb:128 * (kb + 1)], idn[:])
                        nc.vector.tensor_copy(xT[:, :, 128 * i:128 * (i + 1)],
                                              tp[:])

                def layer(rhs, wt, bt, tag, act=True):
                    out = hp.tile([128, 2, CH], mdt, tag=tag)
                    for m in range(2):
                        ps = mmp.tile([128, CH], f32, tag="mm")
                        for k in range(2):
                            for n in range(CH // 512):
                                nc.tensor.matmul(
                                    ps[:, 512 * n:512 * (n + 1)],
                                    wt[:, k, m, :],
                                    rhs[:, k, 512 * n:512 * (n + 1)],
                                    start=(k == 0), stop=(k == 1))
                        if m == 0:
                            nc.scalar.activation(out[:, m, :], ps[:], AF.Relu,
                                                 bias=bt[:, m:m + 1])
                        else:
                            nc.vector.tensor_scalar(out[:, m, :], ps[:],
                                                    bt[:, m:m + 1], 0.0,
                                                    ALU.add, ALU.max)
                    return out

                h1 = layer(xT, w1, b1, "h1")
                h2 = layer(h1, w2, b2, "h2")
                g1 = layer(xT, vw1, vb1, "g1")
                g2 = layer(g1, vw2, vb2, "g2")

                thc = tcp.tile([33, CH], f32, tag="thc")
                for n in range(CH // 512):
                    ps = thp.tile([5, 512], f32, tag="thps")
                    vp = thp.tile([2, 512], f32, tag="vps")
                    for k in range(2):
                        nc.tensor.matmul(ps[:], w3[:, k, 0:5],
                                         h2[:, k, 512 * n:512 * (n + 1)],
                                         start=(k == 0), stop=(k == 1))
                    for k in range(2):
                        nc.tensor.matmul(vp[:], w3[:, k, 5:7],
                                         g2[:, k, 512 * n:512 * (n + 1)],
                                         start=(k == 0), stop=(k == 1))
                    nc.scalar.activation(thc[0:5, 512 * n:512 * (n + 1)], ps[:],
                                         AF.Identity, bias=b3[:])
                    nc.scalar.activation(thc[32:33, 512 * n:512 * (n + 1)],
                                         vp[0:1, :], AF.Identity, bias=b3v[:])
                # theta chunk -> DRAM scratch (contiguous)
                nc.sync.dma_start(th_scr[0:5, CH * c:CH * (c + 1)], thc[0:5, :])
                nc.sync.dma_start(th_scr[5:6, CH * c:CH * (c + 1)],
                                  thc[32:33, :])

        # ---- phase B1: heads (all tensors (128, NT) fp32) ----
        # reshape theta into head layout (row = 128*f + p) via PE transpose
        with (
            tc.tile_pool(name="rs", bufs=2) as rsp,
            tc.tile_pool(name="rp", bufs=2, space=bass.MemorySpace.PSUM) as rpp,
        ):
            for j in range(6):
                tsc = rsp.tile([NT, 128], f32, tag="tsc")
                nc.sync.dma_start(
                    tsc[:], th_scr[j, :].rearrange("(f p) -> f p", p=128))
                pst = rpp.tile([128, NT], f32, tag="pst")
                nc.tensor.transpose(pst[:], tsc[:], idn[0:NT, 0:NT])
                nc.vector.tensor_copy(th[j][:], pst[:])
        V = nc.vector
        S = nc.scalar

        def softplus(dst, src, w):
            V.tensor_scalar(w[0][:], src[:], -5.0, None, ALU.max)      # c
            S.activation(w[1][:], w[0][:], AF.Abs)
            S.activation(w[1][:], w[1][:], AF.Exp, scale=-1.0)         # exp(-|c|)
            S.activation(w[1][:], w[1][:], AF.Ln, bias=c_one[:])       # log1p
            V.tensor_scalar(w[2][:], w[0][:], 0.0, None, ALU.max)      # relu(c)
            V.tensor_add(dst[:], w[1][:], w[2][:])

        softplus(ha, th[3], s2)
        softplus(hb, th[4], s3)

        # v head: sigmoid = 1/(1+exp(-x)); stored, written out in phase C1
        hvs = cp.tile([128, NT], f32)
        S.activation(s[0][:], th[5][:], AF.Exp, scale=-1.0)
        V.tensor_scalar(s[0][:], s[0][:], 1.0, None, ALU.add)
        V.reciprocal(hvs[:], s[0][:])

        # softmax numerators
        V.tensor_max(s[0][:], th[0][:], th[1][:])
        V.tensor_max(s[0][:], s[0][:], th[2][:])
        for j in range(3):
            V.tensor_sub(s[1][:], th[j][:], s[0][:])
            S.activation(he[j][:], s[1][:], AF.Exp)

        def gammaln(dst, z, w):
            # z>0; gammaln via 8-shift + Stirling.  dst may not alias z.
            V.tensor_scalar(w[0][:], z[:], 7.0, None, ALU.add)
            V.tensor_mul(w[0][:], w[0][:], z[:])                       # w=z^2+7z
            V.tensor_scalar(w[1][:], w[0][:], 6.0, None, ALU.add)
            V.tensor_mul(w[1][:], w[1][:], w[0][:])                    # w(w+6)
            V.tensor_scalar(w[2][:], w[0][:], 10.0, None, ALU.add)
            V.tensor_mul(w[1][:], w[1][:], w[2][:])
            V.tensor_scalar(w[2][:], w[0][:], 12.0, None, ALU.add)
            V.tensor_mul(w[1][:], w[1][:], w[2][:])                    # prod
            S.activation(w[1][:], w[1][:], AF.Ln)                      # logprod
            S.activation(w[2][:], z[:], AF.Ln, bias=c_eight[:])        # log(z+8)
            V.tensor_scalar(w[3][:], z[:], 8.0, None, ALU.add)
            V.reciprocal_approx_fast(w[4][:], w[3][:])                 # r=1/(z+8)
            V.tensor_mul(w[3][:], w[4][:], w[4][:])                    # r2
            V.tensor_scalar(w[5][:], w[3][:], STIR3, STIR2, ALU.mult, ALU.add)
            V.tensor_mul(w[5][:], w[5][:], w[3][:])
            V.tensor_scalar(w[5][:], w[5][:], STIR1, None, ALU.add)
            V.tensor_mul(w[5][:], w[5][:], w[4][:])                    # ser
            V.tensor_scalar(w[3][:], z[:], 7.5, None, ALU.add)
            V.tensor_mul(w[3][:], w[3][:], w[2][:])                    # (u-.5)logu
            V.tensor_sub(w[3][:], w[3][:], z[:])
            V.tensor_add(w[3][:], w[3][:], w[5][:])
            V.tensor_scalar(w[3][:], w[3][:], HALF_LN2PI - 8.0, None, ALU.add)
            V.tensor_sub(dst[:], w[3][:], w[1][:])

        ga, gb, gab = s[6], s[7], hnlB
        gammaln(ga, ha, s2)
        gammaln(gb, hb, s3)
        V.tensor_add(hw2[:], ha[:], hb[:])   # hw2 free until phase B2
        gammaln(gab, hw2, s4)
        # hnlB (=gab tile) := gab - ga - gb  == -logB
        V.tensor_sub(hnlB[:], hnlB[:], ga[:])
        V.tensor_sub(hnlB[:], hnlB[:], gb[:])

        def edge(dst, z, zq, x1, w):
            # dst = x1^z / B * sum_{n<NSER} c_n(zq) x1^n / (z+n)
            lnx1 = float(np.log(x1))
            V.reciprocal_approx_fast(w[0][:], z[:])
            V.tensor_copy(w[1][:], w[0][:])                            # S
            first = True
            for n in range(1, NSER):
                V.tensor_scalar(w[2][:], zq[:], -x1 / n, x1, ALU.mult, ALU.add)
                if first:
                    V.tensor_copy(w[3][:], w[2][:])
                    first = False
                else:
                    V.tensor_mul(w[3][:], w[3][:], w[2][:])            # c_n
                V.tensor_scalar(w[4][:], z[:], float(n), None, ALU.add)
                V.reciprocal_approx_fast(w[0][:], w[4][:])
                V.tensor_mul(w[4][:], w[3][:], w[0][:])
                V.tensor_add(w[1][:], w[1][:], w[4][:])
            V.tensor_scalar(w[2][:], z[:], lnx1, None, ALU.mult)
            V.tensor_add(w[2][:], w[2][:], hnlB[:])
            S.activation(w[2][:], w[2][:], AF.Exp)
            V.tensor_mul(dst[:], w[2][:], w[1][:])

        edge(hE1, ha, hb, _X1, s2)
        edge(hE99, hb, ha, _X1B, s3)

        # coefT rows 0,1 = a-1, b-1 (head -> tile-major via cast DMA);
        # bias compensation for the recentered L rows:
        #   hnlB += sh0*(a-1) + sh1*(b-1) + sh2
        sh0, sh1, sh2 = (float(v) for v in _L_SHIFTS)
        V.tensor_scalar(s[0][:], ha[:], 1.0, None, ALU.subtract)
        V.tensor_scalar(s[1][:], hb[:], 1.0, None, ALU.subtract)
        V.tensor_scalar(s[2][:], s[0][:], sh0, sh2, ALU.mult, ALU.add)
        V.tensor_add(hnlB[:], hnlB[:], s[2][:])
        V.tensor_scalar(s[2][:], s[1][:], sh1, None, ALU.mult)
        V.tensor_add(hnlB[:], hnlB[:], s[2][:])

        # ---- phase C1: quadrature bins per row-tile ----
        with (
            tc.tile_pool(name="ep", bufs=4, space=bass.MemorySpace.PSUM) as epp,
            tc.tile_pool(name="gp", bufs=3) as gpp,
        ):
            # head->tile-major transposes: coefT rows (a-1, b-1) and v output
            for r, src in ((0, s[0]), (1, s[1])):
                pc = epp.tile([NT, 128], f32, tag="pc")
                nc.tensor.transpose(pc[:], src[:], idn[:])
                sc = gpp.tile([NT, 128], f32, tag="sc")
                nc.vector.tensor_copy(sc[:], pc[:])
                nc.sync.dma_start(
                    cf_scr[r, :].rearrange("(f p) -> f p", p=128), sc[:])
            nc.gpsimd.dma_start(coefT[0:2, :], cf_scr[:])
            pv = epp.tile([NT, 128], f32, tag="pc")
            nc.tensor.transpose(pv[:], hvs[:], idn[:])
            sv = gpp.tile([NT, 128], f32, tag="sc")
            nc.vector.tensor_copy(sv[:], pv[:])
            nc.sync.dma_start(v_d[:].rearrange("(f p) -> f p", p=128), sv[:])
            for t in range(NT):
                ps = epp.tile([128, EW], f32, tag="E")
                nc.tensor.matmul(ps[:], coefT[:, 128 * t:128 * (t + 1)], Ls[:],
                                 start=True, stop=True)
                G = gpp.tile([128, EW], f32, tag="G")
                nc.scalar.activation(G[:], ps[:], AF.Exp,
                                     bias=hnlB[:, t:t + 1],
                                     accum_out=hsum[:, t:t + 1])
                nc.vector.tensor_reduce(
                    bins[:, NB * t:NB * (t + 1)],
                    G[:].rearrange("p (j q) -> p j q", q=NQ),
                    axis=AX.X, op=ALU.add)

        # ---- phase B2: normalization factors ----
        V.tensor_add(s[0][:], hsum[:], hE1[:])
        V.tensor_add(s[0][:], s[0][:], hE99[:])           # T
        V.tensor_mul(s[1][:], he[2][:], s[0][:])
        V.tensor_add(s[1][:], s[1][:], he[0][:])
        V.tensor_add(s[1][:], s[1][:], he[1][:])          # denom
        V.reciprocal(s[2][:], s[1][:])                    # dr
        V.tensor_mul(hw2[:], he[2][:], s[2][:])
        V.tensor_mul(hp0[:], he[0][:], s[2][:])
        V.tensor_mul(hp1[:], he[1][:], s[2][:])
        V.tensor_mul(hE1[:], hE1[:], hw2[:])
        V.tensor_mul(hE99[:], hE99[:], hw2[:])
        # interleave [p0, E1n, E99n, p1] per tile for cheap C3 copies
        edg = cp.tile([128, NT, 4], f32)
        V.tensor_copy(edg[:, :, 0], hp0[:])
        V.tensor_copy(edg[:, :, 1], hE1[:])
        V.tensor_copy(edg[:, :, 2], hE99[:])
        V.tensor_copy(edg[:, :, 3], hp1[:])

        # ---- phase C3: assemble + store pdf ----
        with tc.tile_pool(name="op", bufs=6) as opp:
            for t in range(NT):
                O = opp.tile([128, 101], f32, tag="O")
                nc.vector.tensor_scalar(O[:, 2:99], bins[:, NB * t:NB * (t + 1)],
                                        hw2[:, t:t + 1], None, ALU.mult)
                nc.vector.tensor_copy(O[:, 0:2], edg[:, t, 0:2])
                nc.scalar.activation(O[:, 99:101], edg[:, t, 2:4], AF.Copy)
                nc.sync.dma_start(pdf_d[128 * t:128 * (t + 1), :], O[:])

    nc.compile()
    return nc


_NC_CACHE = {}


def _get_nc(B_core):
    if B_core not in _NC_CACHE:
        _NC_CACHE[B_core] = build(B_core)
    return _NC_CACHE[B_core]


def make_in_maps(inputs, B_core, n_cores):
    cfpad = np.zeros((6, B_core), np.float32)
    cfpad[0] = 1.0
    ident = np.eye(128, dtype=np.float32)
    maps = []
    for c in range(n_cores):
        m = {"x": np.ascontiguousarray(inputs["x"][B_core * c:B_core * (c + 1)]),
             "Lmat": _L_CONST, "cfpad": cfpad, "ident": ident}
        for nm in ("pW1", "pb1", "pW2", "pb2", "pW3", "pb3",
                   "vW1", "vb1", "vW2", "vb2", "vW3", "vb3"):
            m[nm] = np.asarray(inputs[nm], np.float32)
        maps.append(m)
    return maps


def kernel(**inputs):
    B = inputs["x"].shape[0]
    B_core = B // N_CORES
    nc = _get_nc(B_core)
    maps = make_in_maps(inputs, B_core, N_CORES)
    res = run_bass_kernel_spmd(nc, maps, list(range(N_CORES))).results
    pdf = np.concatenate([r["pdf"] for r in res], axis=0)
    v = np.concatenate([r["v"] for r in res], axis=0)
    return pdf, v


# revision 25
# speedup vs baseline: 1.5618x; 1.1047x over previous
"""Trainium2 Bass kernel for nn_AgentModel_30717606101642.

Computes, for x (B,256) through two small MLPs:
  pdf (B,101) — mixture of [pi0, beta-bin-masses*pi2, pi1], normalized
  v   (B,)    — sigmoid head

Key algorithmic idea: the reference's 41-step Lentz continued fraction for
the beta CDF is only used through *differences* cdf[j+1]-cdf[j].  Those are
bin integrals of the beta density, which we evaluate directly with per-bin
Gauss-Legendre quadrature:
    bin_j = sum_q exp((a-1)*log t_jq + (b-1)*log(1-t_jq) + log w_jq - logB)
The exponent is a K=3 matmul (PE), the exp is one ACT op per row-tile, and
the q-sum is a DVE reduce.  The first/last bins (integrable singularities)
use a 6-term analytic series.  Validated to ~1e-6 absmax against the
reference on the full batch.

Data parallel across 8 cores: batch sharded, weights replicated.
"""
import sys
import os

for _p in ("/opt/trn_rl_repo", "/root/.axon_site/_ro/trn_rl_repo"):
    if os.path.isdir(_p) and _p not in sys.path:
        sys.path.append(_p)

import numpy as np
import concourse.bass as bass
import concourse.bacc as bacc
import concourse.mybir as mybir
import concourse.tile as tile
from concourse.bass_utils import run_bass_kernel_spmd

f32 = mybir.dt.float32
f32r = mybir.dt.float32r
AF = mybir.ActivationFunctionType
ALU = mybir.AluOpType
AX = mybir.AxisListType

N_CORES = 8
B_FULL = 65536
F = 256
NGRID = 100
NQ = 2          # Gauss-Legendre points per interior bin
NB = 97         # interior bins (pdf cols 2..98); bins 0 and 98 via series
EW = NB * NQ    # exponent width per row-tile
NSER = 5        # edge-series terms
USE_F32R = True  # matmul dtype knob

HALF_LN2PI = 0.9189385332046727
STIR1, STIR2, STIR3 = 1.0 / 12.0, -1.0 / 360.0, 1.0 / 1260.0


def host_constants():
    grid = np.linspace(0.0, 1.0, NGRID, dtype=np.float32).astype(np.float64)
    xg, wg = np.polynomial.legendre.leggauss(NQ)
    L = np.zeros((3, NB, NQ))
    for j in range(NB):
        lo, hi = grid[j + 1], grid[j + 2]
        mid, half = 0.5 * (lo + hi), 0.5 * (hi - lo)
        t = mid + half * xg
        L[0, j] = np.log(t)
        L[1, j] = np.log1p(-t)
        L[2, j] = np.log(half * wg)
    # recenter rows to shrink f32r rounding error; compensated in the bias
    shifts = L.reshape(3, -1).mean(-1)
    Lc = np.zeros((8, EW), np.float32)
    Lc[0:3] = (L - shifts[:, None, None]).reshape(3, EW).astype(np.float32)
    x1 = float(grid[1])
    x1b = float(1.0 - np.float32(grid[98]))
    return Lc, shifts, x1, x1b


_L_CONST, _L_SHIFTS, _X1, _X1B = host_constants()


def build(B_core, CH=1024):
    """Emit the per-core program. B_core rows; CH-row chunks for the MLP."""
    assert B_core % CH == 0 and CH % 512 == 0
    NCH = B_core // CH
    NT = B_core // 128          # row-tiles (also head free-dim width)
    TPC = CH // 128             # row-tiles per chunk
    FPC = CH // 128             # head cols per chunk (row = 128*f + p)
    mdt = f32r if USE_F32R else f32

    nc = bacc.Bacc("TRN2", target_bir_lowering=False, debug=False)

    x_d = nc.dram_tensor("x", [B_core, F], f32, kind="ExternalInput")
    wds = {}
    for nm, shp in [("pW1", [F, 256]), ("pW2", [256, 256]), ("pW3", [256, 5]),
                    ("vW1", [F, 256]), ("vW2", [256, 256]), ("vW3", [256, 1])]:
        wds[nm] = nc.dram_tensor(nm, shp, mdt, kind="ExternalInput")
    bds = {}
    for nm, shp in [("pb1", [256]), ("pb2", [256]), ("pb3", [5]),
                    ("vb1", [256]), ("vb2", [256]), ("vb3", [1])]:
        bds[nm] = nc.dram_tensor(nm, shp, f32, kind="ExternalInput")
    L_d = nc.dram_tensor("Lmat", [8, EW], mdt, kind="ExternalInput")
    ones_d = nc.dram_tensor("cfpad", [6, B_core], mdt, kind="ExternalInput")
    id_d = nc.dram_tensor("ident", [128, 128], f32, kind="ExternalInput")
    th_scr = nc.dram_tensor("th_scr", [6, B_core], f32)
    cf_scr = nc.dram_tensor("cf_scr", [2, B_core], f32)
    pdf_d = nc.dram_tensor("pdf", [B_core, 101], f32, kind="ExternalOutput")
    v_d = nc.dram_tensor("v", [B_core], f32, kind="ExternalOutput")

    with tile.TileContext(nc) as tc:
      with tc.tile_pool(name="const", bufs=1) as cp:
        # ---- persistent SBUF ----
        w1 = cp.tile([128, 2, 2, 128], mdt)   # [k, m] tiles of pW1
        w2 = cp.tile([128, 2, 2, 128], mdt)
        vw1 = cp.tile([128, 2, 2, 128], mdt)
        vw2 = cp.tile([128, 2, 2, 128], mdt)
        w3 = cp.tile([128, 2, 8], mdt)        # [k, {theta0..4, v, pad}]
        b1 = cp.tile([128, 2], f32)
        b2 = cp.tile([128, 2], f32)
        vb1 = cp.tile([128, 2], f32)
        vb2 = cp.tile([128, 2], f32)
        b3 = cp.tile([5, 1], f32)
        b3v = cp.tile([1, 1], f32)
        Ls = cp.tile([8, EW], mdt)
        idn = cp.tile([128, 128], f32)
        coefT = cp.tile([8, B_core], mdt)
        bins = cp.tile([128, NT * NB], f32)
        th = [cp.tile([128, NT], f32, name=f"th{j}") for j in range(6)]   # theta comps, head layout
        # head-phase values
        ha = cp.tile([128, NT], f32)
        hb = cp.tile([128, NT], f32)
        hnlB = cp.tile([128, NT], f32)     # -logB - shift-compensation
        he = [cp.tile([128, NT], f32, name=f"he{j}") for j in range(3)]
        hE1 = cp.tile([128, NT], f32)
        hE99 = cp.tile([128, NT], f32)
        hsum = cp.tile([128, NT], f32)
        hw2 = cp.tile([128, NT], f32)
        hp0 = cp.tile([128, NT], f32)
        hp1 = cp.tile([128, NT], f32)
        s = [cp.tile([128, NT], f32, name=f"hs{j}") for j in range(8)]    # scratch
        s2 = [cp.tile([128, NT], f32, name=f"hs2_{j}") for j in range(6)]
        s3 = [cp.tile([128, NT], f32, name=f"hs3_{j}") for j in range(6)]
        s4 = [cp.tile([128, NT], f32, name=f"hs4_{j}") for j in range(6)]
        s5 = [cp.tile([128, NT], f32, name=f"hs5_{j}") for j in range(6)]
        c_one = cp.tile([128, 1], f32)
        c_eight = cp.tile([128, 1], f32)
        nc.vector.memset(c_one[:], 1.0)
        nc.vector.memset(c_eight[:], 8.0)

        # ---- weight/const loads ----
        nc.sync.dma_start(idn[:], id_d[:])
        for bt, dname in [(b1, "pb1"), (b2, "pb2"), (vb1, "vb1"), (vb2, "vb2")]:
            for m in range(2):
                nc.sync.dma_start(
                    bt[:, m:m + 1],
                    bds[dname][128 * m:128 * (m + 1)].rearrange("(p o) -> p o", o=1))
        nc.sync.dma_start(b3[:], bds["pb3"][:].rearrange("(p o) -> p o", o=1))
        nc.sync.dma_start(b3v[:], bds["vb3"][:].rearrange("(p o) -> p o", o=1))
        for wt, dname in [(w1, "pW1"), (w2, "pW2"), (vw1, "vW1"), (vw2, "vW2")]:
            eng = nc.sync if wt is w1 else nc.gpsimd
            for k in range(2):
                for m in range(2):
                    eng.dma_start(wt[:, k, m, :],
                                  wds[dname][128 * k:128 * (k + 1),
                                             128 * m:128 * (m + 1)])
        for k in range(2):
            nc.gpsimd.dma_start(w3[:, k, 0:5], wds["pW3"][128 * k:128 * (k + 1), :])
            nc.gpsimd.dma_start(w3[:, k, 5:6], wds["vW3"][128 * k:128 * (k + 1), :])
        nc.gpsimd.dma_start(Ls[:], L_d[:])
        nc.gpsimd.dma_start(coefT[2:8, :], ones_d[:])

        # ---- phase A: transpose x + MLPs, chunked over rows ----
        with (
            tc.tile_pool(name="xl", bufs=3) as xlp,
            tc.tile_pool(name="hc", bufs=2) as hp,
            tc.tile_pool(name="tc1", bufs=2) as tcp,
            tc.tile_pool(name="tp", bufs=2, space=bass.MemorySpace.PSUM) as tpp,
            tc.tile_pool(name="mm", bufs=2, space=bass.MemorySpace.PSUM) as mmp,
            tc.tile_pool(name="thp", bufs=1, space=bass.MemorySpace.PSUM) as thp,
        ):
            for c in range(NCH):
                xT = hp.tile([128, 2, CH], mdt, tag="xT")
                for g in range(TPC // 4):
                    t0 = TPC * c + 4 * g
                    xg = xlp.tile([128, 4, F], f32, tag="xl")
                    nc.sync.dma_start(
                        xg[:],
                        x_d[128 * t0:128 * t0 + 512, :].rearrange(
                            "(u p) f -> p u f", p=128))
                    for u in range(4):
                        i = 4 * g + u
                        tp = tpp.tile([128, 2, 128], f32, tag="tp")
                        for kb in range(2):
                            nc.tensor.transpose(
                                tp[:, kb, :],
                                xg[:, u, 128 * kb:128 * (kb + 1)], idn[:])
                        nc.vector.tensor_copy(xT[:, :, 128 * i:128 * (i + 1)],
                                              tp[:])

                def layer(rhs, wt, bt, tag, act=True):
                    out = hp.tile([128, 2, CH], mdt, tag=tag)
                    for m in range(2):
                        ps = mmp.tile([128, CH], f32, tag="mm")
                        for k in range(2):
                            for n in range(CH // 512):
                                nc.tensor.matmul(
                                    ps[:, 512 * n:512 * (n + 1)],
                                    wt[:, k, m, :],
                                    rhs[:, k, 512 * n:512 * (n + 1)],
                                    start=(k == 0), stop=(k == 1))
                        if m == 0:
                            nc.scalar.activation(out[:, m, :], ps[:], AF.Relu,
                                                 bias=bt[:, m:m + 1])
                        else:
                            nc.vector.tensor_scalar(out[:, m, :], ps[:],
                                                    bt[:, m:m + 1], 0.0,
                                                    ALU.add, ALU.max)
                    return out

                h1 = layer(xT, w1, b1, "h1")
                h2 = layer(h1, w2, b2, "h2")
                g1 = layer(xT, vw1, vb1, "g1")
                g2 = layer(g1, vw2, vb2, "g2")

                thc = tcp.tile([33, CH], f32, tag="thc")
                for n in range(CH // 512):
                    ps = thp.tile([5, 512], f32, tag="thps")
                    vp = thp.tile([2, 512], f32, tag="vps")
                    for k in range(2):
                        nc.tensor.matmul(ps[:], w3[:, k, 0:5],
                                         h2[:, k, 512 * n:512 * (n + 1)],
                                         start=(k == 0), stop=(k == 1))
                    for k in range(2):
                        nc.tensor.matmul(vp[:], w3[:, k, 5:7],
                                         g2[:, k, 512 * n:512 * (n + 1)],
                                         start=(k == 0), stop=(k == 1))
                    nc.scalar.activation(thc[0:5, 512 * n:512 * (n + 1)], ps[:],
                                         AF.Identity, bias=b3[:])
                    nc.scalar.activation(thc[32:33, 512 * n:512 * (n + 1)],
                                         vp[0:1, :], AF.Identity, bias=b3v[:])
                # theta chunk -> DRAM scratch (contiguous)
                nc.sync.dma_start(th_scr[0:5, CH * c:CH * (c + 1)], thc[0:5, :])
                nc.sync.dma_start(th_scr[5:6, CH * c:CH * (c + 1)],
                                  thc[32:33, :])

        # ---- phase B1: heads (all tensors (128, NT) fp32) ----
        # reshape theta into head layout (row = 128*f + p) via PE transpose
        with (
            tc.tile_pool(name="rs", bufs=2) as rsp,
            tc.tile_pool(name="rp", bufs=2, space=bass.MemorySpace.PSUM) as rpp,
        ):
            for j in range(6):
                tsc = rsp.tile([NT, 128], f32, tag="tsc")
                nc.sync.dma_start(
                    tsc[:], th_scr[j, :].rearrange("(f p) -> f p", p=128))
                pst = rpp.tile([128, NT], f32, tag="pst")
                nc.tensor.transpose(pst[:], tsc[:], idn[0:NT, 0:NT])
                nc.vector.tensor_copy(th[j][:], pst[:])
        V = nc.vector
        S = nc.scalar

        def softplus(dst, src, w):
            V.tensor_scalar(w[0][:], src[:], -5.0, None, ALU.max)      # c
            S.activation(w[1][:], w[0][:], AF.Abs)
            S.activation(w[1][:], w[1][:], AF.Exp, scale=-1.0)         # exp(-|c|)
            S.activation(w[1][:], w[1][:], AF.Ln, bias=c_one[:])       # log1p
            V.tensor_scalar(w[2][:], w[0][:], 0.0, None, ALU.max)      # relu(c)
            V.tensor_add(dst[:], w[1][:], w[2][:])

        softplus(ha, th[3], s2)
        softplus(hb, th[4], s3)

        # v head: sigmoid = 1/(1+exp(-x)); stored, written out in phase C1
        hvs = cp.tile([128, NT], f32)
        S.activation(s[0][:], th[5][:], AF.Exp, scale=-1.0)
        V.tensor_scalar(s[0][:], s[0][:], 1.0, None, ALU.add)
        V.reciprocal(hvs[:], s[0][:])

        # softmax numerators
        V.tensor_max(s[0][:], th[0][:], th[1][:])
        V.tensor_max(s[0][:], s[0][:], th[2][:])
        for j in range(3):
            V.tensor_sub(s[1][:], th[j][:], s[0][:])
            S.activation(he[j][:], s[1][:], AF.Exp)

        def gammaln(dst, z, w):
            # z>0; gammaln via 8-shift + Stirling.  dst may not alias z.
            V.tensor_scalar(w[0][:], z[:], 7.0, None, ALU.add)
            V.tensor_mul(w[0][:], w[0][:], z[:])                       # w=z^2+7z
            V.tensor_scalar(w[1][:], w[0][:], 6.0, None, ALU.add)
            V.tensor_mul(w[1][:], w[1][:], w[0][:])                    # w(w+6)
            V.tensor_scalar(w[2][:], w[0][:], 10.0, None, ALU.add)
            V.tensor_mul(w[1][:], w[1][:], w[2][:])
            V.tensor_scalar(w[2][:], w[0][:], 12.0, None, ALU.add)
            V.tensor_mul(w[1][:], w[1][:], w[2][:])                    # prod
            S.activation(w[1][:], w[1][:], AF.Ln)                      # logprod
            S.activation(w[2][:], z[:], AF.Ln, bias=c_eight[:])        # log(z+8)
            V.tensor_scalar(w[3][:], z[:], 8.0, None, ALU.add)
            V.reciprocal_approx_fast(w[4][:], w[3][:])                 # r=1/(z+8)
            V.tensor_mul(w[3][:], w[4][:], w[4][:])                    # r2
            V.tensor_scalar(w[5][:], w[3][:], STIR3, STIR2, ALU.mult, ALU.add)
            V.tensor_mul(w[5][:], w[5][:], w[3][:])
            V.tensor_scalar(w[5][:], w[5][:], STIR1, None, ALU.add)
            V.tensor_mul(w[5][:], w[5][:], w[4][:])                    # ser
            V.tensor_scalar(w[3][:], z[:], 7.5, None, ALU.add)
            V.tensor_mul(w[3][:], w[3][:], w[2][:])                    # (u-.5)logu
            V.tensor_sub(w[3][:], w[3][:], z[:])
            V.tensor_add(w[3][:], w[3][:], w[5][:])
            V.tensor_scalar(w[3][:], w[3][:], HALF_LN2PI - 8.0, None, ALU.add)
            V.tensor_sub(dst[:], w[3][:], w[1][:])

        ga, gb, gab = s[6], s[7], hnlB
        gammaln(ga, ha, s2)
        gammaln(gb, hb, s3)
        V.tensor_add(hw2[:], ha[:], hb[:])   # hw2 free until phase B2
        gammaln(gab, hw2, s4)
        # hnlB (=gab tile) := gab - ga - gb  == -logB
        V.tensor_sub(hnlB[:], hnlB[:], ga[:])
        V.tensor_sub(hnlB[:], hnlB[:], gb[:])

        def edge(dst, z, zq, x1, w):
            # dst = x1^z / B * sum_{n<NSER} c_n(zq) x1^n / (z+n)
            lnx1 = float(np.log(x1))
            V.reciprocal_approx_fast(w[0][:], z[:])
            V.tensor_copy(w[1][:], w[0][:])                            # S
            first = True
            for n in range(1, NSER):
                V.tensor_scalar(w[2][:], zq[:], -x1 / n, x1, ALU.mult, ALU.add)
                if first:
                    V.tensor_copy(w[3][:], w[2][:])
                    first = False
                else:
                    V.tensor_mul(w[3][:], w[3][:], w[2][:])            # c_n
                V.tensor_scalar(w[4][:], z[:], float(n), None, ALU.add)
                V.reciprocal_approx_fast(w[0][:], w[4][:])
                V.tensor_mul(w[4][:], w[3][:], w[0][:])
                V.tensor_add(w[1][:], w[1][:], w[4][:])
            V.tensor_scalar(w[2][:], z[:], lnx1, None, ALU.mult)
            V.tensor_add(w[2][:], w[2][:], hnlB[:])
            S.activation(w[2][:], w[2][:], AF.Exp)
            V.tensor_mul(dst[:], w[2][:], w[1][:])

        edge(hE1, ha, hb, _X1, s2)
        edge(hE99, hb, ha, _X1B, s3)

        # coefT rows 0,1 = a-1, b-1 (head -> tile-major via cast DMA);
        # bias compensation for the recentered L rows:
        #   hnlB += sh0*(a-1) + sh1*(b-1) + sh2
        sh0, sh1, sh2 = (float(v) for v in _L_SHIFTS)
        V.tensor_scalar(s[0][:], ha[:], 1.0, None, ALU.subtract)
        V.tensor_scalar(s[1][:], hb[:], 1.0, None, ALU.subtract)
        V.tensor_scalar(s[2][:], s[0][:], sh0, sh2, ALU.mult, ALU.add)
        V.tensor_add(hnlB[:], hnlB[:], s[2][:])
        V.tensor_scalar(s[2][:], s[1][:], sh1, None, ALU.mult)
        V.tensor_add(hnlB[:], hnlB[:], s[2][:])

        # ---- phase C1: quadrature bins per row-tile ----
        with (
            tc.tile_pool(name="ep", bufs=4, space=bass.MemorySpace.PSUM) as epp,
            tc.tile_pool(name="gp", bufs=3) as gpp,
        ):
            # head->tile-major transposes: coefT rows (a-1, b-1) and v output
            for r, src in ((0, s[0]), (1, s[1])):
                pc = epp.tile([NT, 128], f32, tag="pc")
                nc.tensor.transpose(pc[:], src[:], idn[:])
                sc = gpp.tile([NT, 128], f32, tag="sc")
                nc.vector.tensor_copy(sc[:], pc[:])
                nc.sync.dma_start(
                    cf_scr[r, :].rearrange("(f p) -> f p", p=128), sc[:])
            nc.gpsimd.dma_start(coefT[0:2, :], cf_scr[:])
            pv = epp.tile([NT, 128], f32, tag="pc")
            nc.tensor.transpose(pv[:], hvs[:], idn[:])
            sv = gpp.tile([NT, 128], f32, tag="sc")
            nc.vector.tensor_copy(sv[:], pv[:])
            nc.sync.dma_start(v_d[:].rearrange("(f p) -> f p", p=128), sv[:])
            for t in range(NT):
                ps = epp.tile([128, EW], f32, tag="E")
                nc.tensor.matmul(ps[:], coefT[:, 128 * t:128 * (t + 1)], Ls[:],
                                 start=True, stop=True)
                G = gpp.tile([128, EW], f32, tag="G")
                nc.scalar.activation(G[:], ps[:], AF.Exp,
                                     bias=hnlB[:, t:t + 1],
                                     accum_out=hsum[:, t:t + 1])
                nc.vector.tensor_reduce(
                    bins[:, NB * t:NB * (t + 1)],
                    G[:].rearrange("p (j q) -> p j q", q=NQ),
                    axis=AX.X, op=ALU.add)

        # ---- phase B2: normalization factors ----
        V.tensor_add(s[0][:], hsum[:], hE1[:])
        V.tensor_add(s[0][:], s[0][:], hE99[:])           # T
        V.tensor_mul(s[1][:], he[2][:], s[0][:])
        V.tensor_add(s[1][:], s[1][:], he[0][:])
        V.tensor_add(s[1][:], s[1][:], he[1][:])          # denom
        V.reciprocal(s[2][:], s[1][:])                    # dr
        V.tensor_mul(hw2[:], he[2][:], s[2][:])
        V.tensor_mul(hp0[:], he[0][:], s[2][:])
        V.tensor_mul(hp1[:], he[1][:], s[2][:])
        V.tensor_mul(hE1[:], hE1[:], hw2[:])
        V.tensor_mul(hE99[:], hE99[:], hw2[:])
        # interleave [p0, E1n, E99n, p1] per tile for cheap C3 copies
        edg = cp.tile([128, NT, 4], f32)
        V.tensor_copy(edg[:, :, 0], hp0[:])
        V.tensor_copy(edg[:, :, 1], hE1[:])
        V.tensor_copy(edg[:, :, 2], hE99[:])
        V.tensor_copy(edg[:, :, 3], hp1[:])

        # ---- phase C3: assemble + store pdf ----
        OB = 4
        with tc.tile_pool(name="op", bufs=4) as opp:
            for t0 in range(0, NT, OB):
                O = opp.tile([128, OB, 101], f32, tag="O")
                for u in range(OB):
                    t = t0 + u
                    nc.vector.tensor_scalar(O[:, u, 2:99],
                                            bins[:, NB * t:NB * (t + 1)],
                                            hw2[:, t:t + 1], None, ALU.mult)
                nc.vector.tensor_copy(O[:, :, 0:2], edg[:, t0:t0 + OB, 0:2])
                nc.scalar.activation(O[:, :, 99:101], edg[:, t0:t0 + OB, 2:4],
                                     AF.Copy)
                nc.sync.dma_start(
                    pdf_d[128 * t0:128 * (t0 + OB), :].rearrange(
                        "(u p) c -> p u c", p=128),
                    O[:])

    nc.compile()
    return nc


_NC_CACHE = {}


def _get_nc(B_core):
    if B_core not in _NC_CACHE:
        _NC_CACHE[B_core] = build(B_core)
    return _NC_CACHE[B_core]


def make_in_maps(inputs, B_core, n_cores):
    cfpad = np.zeros((6, B_core), np.float32)
    cfpad[0] = 1.0
    ident = np.eye(128, dtype=np.float32)
    maps = []
    for c in range(n_cores):
        m = {"x": np.ascontiguousarray(inputs["x"][B_core * c:B_core * (c + 1)]),
             "Lmat": _L_CONST, "cfpad": cfpad, "ident": ident}
        for nm in ("pW1", "pb1", "pW2", "pb2", "pW3", "pb3",
                   "vW1", "vb1", "vW2", "vb2", "vW3", "vb3"):
            m[nm] = np.asarray(inputs[nm], np.float32)
        maps.append(m)
    return maps


def kernel(**inputs):
    B = inputs["x"].shape[0]
    B_core = B // N_CORES
    nc = _get_nc(B_core)
    maps = make_in_maps(inputs, B_core, N_CORES)
    res = run_bass_kernel_spmd(nc, maps, list(range(N_CORES))).results
    pdf = np.concatenate([r["pdf"] for r in res], axis=0)
    v = np.concatenate([r["v"] for r in res], axis=0)
    return pdf, v
